# revision 22
# baseline (speedup 1.0000x reference)
"""Trainium2 Bass kernel for BasicInteractionNetworkModule.

Data-parallel over batch (B=16) across 8 NeuronCores, 2 batches/core.

Math (per batch b):
  senders   = S^T @ O          [R, 128]   (S = sender_relations [128, R])
  receivers = R_rel^T @ O      [R, 128]
  rel_x = [senders, receivers, info]   [R, 320]
  h = relu-MLP(rel_x): 320 -> 256 -> 256 -> 256 -> 128 (relu after every layer)
  eff_recv = R_rel @ effects   [128, 128]
  obj_x = [O, ext, eff_recv]   [128, 288]
  out = relu-MLP2(obj_x): 288 -> 256 -> 256 -> 128 (no final relu)

Device strategy: relation-MLP activations feature-major (partition = feature)
so every layer is out^T = W^T @ H^T with the moving operand streaming 512-col
slabs at 1 col/cycle, all in bf16. Layer-1 folds the sender/receiver
projections via host-precomputed A_s = O @ rw1[:128], A_r = O @ rw1[128:256].
Three-stage software pipeline per chunk k: A(k)=L1/L2/L3, B(k-1)=L4,
C(k-2)=aggregation, so every PSUM-evac -> stationary-reload handoff has a
full chunk of slack. Each 512-col PSUM slab is one bank (psB rotates 5).
L4's bias is folded into the evacuation (DVE add, ACT relu). The aggregation
accumulates all 127 rel-blocks of a batch into a persistent PSUM region.
The object MLP's objects/ext contributions are host-precomputed (G1pre).
"""

import numpy as np
import ml_dtypes

B, N_OBJ, N_REL = 16, 128, 16256
OBJ_D, REL_D, EFF_D, EXT_D, OUT_D = 128, 64, 128, 32, 128
HID = 256
N_CORES = 8
B_CORE = B // N_CORES  # 2
M_CHUNK = 1024

_CACHE = {}

# column offsets in the shared bf16 weight blob
_BF = {}
_off = 0
for _name, _n in [("rw1i", 256), ("rw2", 512), ("rw3", 512), ("rw4", 256),
                  ("ow1e", 256), ("ow2", 512), ("ow3", 256), ("B4", 512)]:
    _BF[_name] = (_off, _n)
    _off += _n
BF_COLS = _off
# per-core bf16 blob: As/Ar per batch
_PB = {}
_off = 0
for _name, _n in [("As0", 256), ("Ar0", 256), ("As1", 256), ("Ar1", 256)]:
    _PB[_name] = (_off, _n)
    _off += _n
PB_COLS = _off
# shared f32 blob
_F32 = {}
_off = 0
for _name, _n in [("rb1", 2), ("rb2", 2), ("rb3", 2), ("ob2", 2), ("ob3r", 128)]:
    _F32[_name] = (_off, _n)
    _off += _n
F32_COLS = _off
# per-core f32 blob: G1pre per batch [128, 2, 128]
_PF = {"G1pre0": (0, 256), "G1pre1": (256, 256)}
PF_COLS = 512


def _chunks():
    out = []
    base = 0
    while base < N_REL:
        mc = min(M_CHUNK, N_REL - base)
        out.append((base, mc))
        base += mc
    return out


def _mtiles(mc):
    out = []
    base = 0
    while base < mc:
        n = min(512, mc - base)
        out.append((base, n))
        base += n
    return out


def build_kernel():
    from concourse import bacc
    import concourse.mybir as mybir
    import concourse.tile as tile

    F32 = mybir.dt.float32
    BF16 = mybir.dt.bfloat16
    RELU = mybir.ActivationFunctionType.Relu
    ADD = mybir.AluOpType.add
    MAX = mybir.AluOpType.max
    MULT = mybir.AluOpType.mult

    nc = bacc.Bacc(None)

    S_d = nc.dram_tensor("s_rel", [B_CORE, N_OBJ, N_REL], BF16, kind="ExternalInput")
    R_d = nc.dram_tensor("r_rel", [B_CORE, N_OBJ, N_REL], BF16, kind="ExternalInput")
    IT_d = nc.dram_tensor("info_t", [B_CORE, REL_D, N_REL], BF16, kind="ExternalInput")
    # receiver_relations pre-permuted on host: [b, p, s, o] = R^T[b, s*128+p, o]
    # so each partition's per-chunk read is one contiguous run
    RT_d = nc.dram_tensor("r_rel_t", [B_CORE, 128, N_REL // 128, N_OBJ], BF16,
                          kind="ExternalInput")

    WB_d = nc.dram_tensor("wblob_bf", [128, BF_COLS], BF16, kind="ExternalInput")
    PB_d = nc.dram_tensor("pblob_bf", [128, PB_COLS], BF16, kind="ExternalInput")
    WF_d = nc.dram_tensor("wblob_f32", [128, F32_COLS], F32, kind="ExternalInput")
    PF_d = nc.dram_tensor("pblob_f32", [128, PF_COLS], F32, kind="ExternalInput")

    out_d = nc.dram_tensor("out", [B_CORE, N_OBJ, OUT_D], F32, kind="ExternalOutput")

    with tile.TileContext(nc) as tc:
        with (
            tc.tile_pool(name="wts", bufs=1) as wts,
            tc.tile_pool(name="perb", bufs=2) as perb,
            tc.tile_pool(name="cin", bufs=6) as cin,
            tc.tile_pool(name="acts", bufs=3) as acts,
            tc.tile_pool(name="psB", bufs=5, space="PSUM") as psB,
            tc.tile_pool(name="ps4", bufs=2, space="PSUM") as ps4p,
            tc.tile_pool(name="psE", bufs=1, space="PSUM") as psE,
        ):
            WB = wts.tile([128, BF_COLS], BF16)
            PBt = wts.tile([128, PB_COLS], BF16)
            WF = wts.tile([128, F32_COLS], F32)
            PF = wts.tile([128, PF_COLS], F32)

            def bf(name):
                o, n = _BF[name]
                return WB[:, o:o + n]

            def pb(name):
                o, n = _PB[name]
                return PBt[:, o:o + n]

            def f32(name):
                o, n = _F32[name]
                return WF[:, o:o + n]

            _o, _n = _BF["rw1i"]
            rw1i = WB[:64, _o:_o + _n].rearrange("p (two h) -> p two h", two=2)
            rw2 = bf("rw2").rearrange("p (two h) -> p two h", two=2)
            rw3 = bf("rw3").rearrange("p (two h) -> p two h", two=2)
            rw4 = bf("rw4").rearrange("p (two h) -> p two h", two=2)
            ow1e = bf("ow1e").rearrange("p (two h) -> p two h", two=2)
            ow2 = bf("ow2").rearrange("p (two h) -> p two h", two=2)
            ow3 = bf("ow3").rearrange("p (two h) -> p two h", two=2)
            B4 = bf("B4")
            rb1 = f32("rb1")
            rb2 = f32("rb2")
            rb3 = f32("rb3")
            ob2 = f32("ob2")
            ob3r = f32("ob3r")
            As = [pb("As0"), pb("As1")]
            Ar = [pb("Ar0"), pb("Ar1")]
            G1pre = [PF[:, 0:256].rearrange("p (two h) -> p two h", two=2),
                     PF[:, 256:512].rearrange("p (two h) -> p two h", two=2)]

            chs = _chunks()
            n_ch = len(chs)
            effaccs = {}
            effacc_all = psE.tile([128, B_CORE * N_OBJ], F32, tag="effacc")
            for b in range(B_CORE):
                effaccs[b] = effacc_all[:, b * N_OBJ:(b + 1) * N_OBJ]

            def dmaA(b, ci, base, mc, st):
                ns = mc // 128
                S_c = cin.tile([N_OBJ, M_CHUNK], BF16, tag="S_c")
                R_c = cin.tile([N_OBJ, M_CHUNK], BF16, tag="R_c")
                I_c = cin.tile([REL_D, M_CHUNK], BF16, tag="I_c")
                RT_c = cin.tile([128, M_CHUNK // 128, N_OBJ], BF16, tag="RT_c")
                nc.sync.dma_start(S_c[:, :mc], S_d[b, :, base:base + mc])
                nc.sync.dma_start(I_c[:, :mc], IT_d[b, :, base:base + mc])
                nc.gpsimd.dma_start(R_c[:, :mc], R_d[b, :, base:base + mc])
                s0 = base // 128
                nc.gpsimd.dma_start(RT_c[:, :ns, :], RT_d[b, :, s0:s0 + ns, :])
                st['S_c'] = S_c; st['R_c'] = R_c; st['I_c'] = I_c; st['RT_c'] = RT_c

            def stageA(b, ci, base, mc, st):
                """Returns a list of slab-group closures (stall-prone)."""
                S_c = st['S_c']; R_c = st['R_c']; I_c = st['I_c']
                H1 = acts.tile([128, 2, M_CHUNK], BF16, tag="H1")
                H2 = acts.tile([128, 2, M_CHUNK], BF16, tag="H2")
                H3 = acts.tile([128, 2, M_CHUNK], BF16, tag="H3")
                st['H3'] = H3
                groups = []

                h0, h1 = slice(0, 128), slice(128, 256)

                def l1(sl, n):
                    # both halves' accumulation chains interleaved over two
                    # psum banks: a bank gets a full stream-time between its
                    # chained read-modify-writes, hiding the write drain
                    def go():
                        pa = psB.tile([128, 512], F32, tag="ps")
                        pb2 = psB.tile([128, 512], F32, tag="ps")
                        nc.tensor.matmul(pa[:, :n], rw1i[:, 0, :], I_c[:, sl], start=True, stop=False)
                        nc.tensor.matmul(pb2[:, :n], rw1i[:, 1, :], I_c[:, sl], start=True, stop=False)
                        nc.tensor.matmul(pa[:, :n], As[b][:, h0], S_c[:, sl], start=False, stop=False)
                        nc.tensor.matmul(pb2[:, :n], As[b][:, h1], S_c[:, sl], start=False, stop=False)
                        nc.tensor.matmul(pa[:, :n], Ar[b][:, h0], R_c[:, sl], start=False, stop=True)
                        nc.tensor.matmul(pb2[:, :n], Ar[b][:, h1], R_c[:, sl], start=False, stop=True)
                        nc.scalar.activation(H1[:, 0, sl], pa[:, :n], RELU,
                                             bias=rb1[:, 0:1], scale=1.0)
                        nc.scalar.activation(H1[:, 1, sl], pb2[:, :n], RELU,
                                             bias=rb1[:, 1:2], scale=1.0)
                    return go

                def l23(sl, n, w, Hin, Hout, rb):
                    def go():
                        pa = psB.tile([128, 512], F32, tag="ps")
                        pb2 = psB.tile([128, 512], F32, tag="ps")
                        nc.tensor.matmul(pa[:, :n], w[:, 0, h0], Hin[:, 0, sl], start=True, stop=False)
                        nc.tensor.matmul(pb2[:, :n], w[:, 0, h1], Hin[:, 0, sl], start=True, stop=False)
                        nc.tensor.matmul(pa[:, :n], w[:, 1, h0], Hin[:, 1, sl], start=False, stop=True)
                        nc.tensor.matmul(pb2[:, :n], w[:, 1, h1], Hin[:, 1, sl], start=False, stop=True)
                        nc.scalar.activation(Hout[:, 0, sl], pa[:, :n], RELU,
                                             bias=rb[:, 0:1], scale=1.0)
                        nc.vector.tensor_scalar(Hout[:, 1, sl], pb2[:, :n],
                                                rb[:, 1:2], 0.0, ADD, MAX)
                    return go

                for mt, n in _mtiles(mc):
                    groups.append(l1(slice(mt, mt + n), n))
                for mt, n in _mtiles(mc):
                    groups.append(l23(slice(mt, mt + n), n, rw2, H1, H2, rb2))
                for mt, n in _mtiles(mc):
                    groups.append(l23(slice(mt, mt + n), n, rw3, H2, H3, rb3))
                return groups

            def stageB(b, ci, base, mc, st):
                """Returns closures of slack-rich L4 work (never stalls)."""
                ns = mc // 128
                H3 = st['H3']
                E3t = acts.tile([128, M_CHUNK], BF16, tag="E3t")
                E3 = acts.tile([128, M_CHUNK], BF16, tag="E3")
                st['E3'] = E3
                fills = []
                for g in range(0, ns, 4):
                    ge = min(g + 4, ns)

                    def l4(g, ge):
                        def go():
                            span = (ge - g) * 128
                            gsl = slice(g * 128, g * 128 + span)
                            ps4 = ps4p.tile([128, 512], F32, tag="ps4")
                            for sj in range(g, ge):
                                sl = slice(sj * 128, (sj + 1) * 128)
                                psl = slice((sj - g) * 128, (sj - g + 1) * 128)
                                nc.tensor.matmul(ps4[:, psl], H3[:, 0, sl], rw4[:, 0, :],
                                                 start=True, stop=False, skip_group_check=True)
                                nc.tensor.matmul(ps4[:, psl], H3[:, 1, sl], rw4[:, 1, :],
                                                 start=False, stop=True, skip_group_check=True)
                            nc.vector.scalar_tensor_tensor(E3t[:, gsl], ps4[:, :span], 1.0,
                                                           B4[:, :span], MULT, ADD)
                            nc.vector.tensor_scalar_max(E3[:, gsl], E3t[:, gsl], 0.0)
                        return go

                    fills.append(l4(g, ge))
                return fills

            def stageC(b, ci, base, mc, st):
                """Returns closures of slack-rich aggregation work."""
                ns = mc // 128
                E3 = st['E3']; RT_c = st['RT_c']
                effacc = effaccs[b]

                def agg(s0, s1):
                    def go():
                        for sj in range(s0, s1):
                            nc.tensor.matmul(effacc, E3[:, sj * 128:(sj + 1) * 128],
                                             RT_c[:, sj, :],
                                             start=(ci == 0 and sj == 0),
                                             stop=(ci == n_ch - 1 and sj == ns - 1),
                                             skip_group_check=True)
                    return go

                mid = (ns + 1) // 2
                return [agg(0, mid), agg(mid, ns)]

            def obj_mlp(b):
                effacc = effaccs[b]
                effTb = perb.tile([128, N_OBJ], BF16, tag="effTb")
                nc.vector.tensor_copy(effTb, effacc)
                G1 = perb.tile([128, 2, N_OBJ], BF16, tag="G1")
                G2 = perb.tile([128, 2, N_OBJ], BF16, tag="G2")
                Gt = perb.tile([128, N_OBJ], F32, tag="Gt")
                for p2 in range(2):
                    ps = ps4p.tile([128, 512], F32, tag="ps4")
                    nc.tensor.matmul(ps[:, :N_OBJ], ow1e[:, p2, :], effTb, start=True, stop=True)
                    nc.vector.scalar_tensor_tensor(Gt, ps[:, :N_OBJ], 1.0,
                                                   G1pre[b][:, p2, :], MULT, ADD)
                    nc.scalar.activation(G1[:, p2, :], Gt, RELU)
                for p2 in range(2):
                    ps = ps4p.tile([128, 512], F32, tag="ps4")
                    h = slice(p2 * 128, (p2 + 1) * 128)
                    nc.tensor.matmul(ps[:, :N_OBJ], ow2[:, 0, h], G1[:, 0, :], start=True, stop=False)
                    nc.tensor.matmul(ps[:, :N_OBJ], ow2[:, 1, h], G1[:, 1, :], start=False, stop=True)
                    nc.scalar.activation(G2[:, p2, :], ps[:, :N_OBJ], RELU,
                                         bias=ob2[:, p2:p2 + 1], scale=1.0)
                ps = ps4p.tile([128, 512], F32, tag="ps4")
                nc.tensor.matmul(ps[:, :OUT_D], G2[:, 0, :], ow3[:, 0, :], start=True, stop=False)
                nc.tensor.matmul(ps[:, :OUT_D], G2[:, 1, :], ow3[:, 1, :], start=False, stop=True)
                ob = perb.tile([N_OBJ, OUT_D], F32, tag="ob")
                nc.vector.tensor_tensor(ob, ps[:, :OUT_D], ob3r, ADD)
                nc.sync.dma_start(out_d[b], ob)

            flat = [(b, ci, base, mc) for b in range(B_CORE)
                    for ci, (base, mc) in enumerate(chs)]
            n_flat = len(flat)
            sts = [dict() for _ in flat]

            # critical small weights first, then chunk-0/1 streams, then the
            # bulk weights split across both rings; loop prefetches 2 chunks
            nc.sync.dma_start(WF, WF_d[:])
            nc.sync.dma_start(WB[:, 0:256], WB_d[:, 0:256])        # rw1i
            nc.gpsimd.dma_start(PBt, PB_d[:])                      # As/Ar
            dmaA(*flat[0][:4], sts[0])
            nc.sync.dma_start(WB[:, 256:1280], WB_d[:, 256:1280])  # rw2,rw3
            dmaA(*flat[1][:4], sts[1])
            nc.gpsimd.dma_start(WB[:, 1280:BF_COLS], WB_d[:, 1280:BF_COLS])
            nc.gpsimd.dma_start(PF, PF_d[:])

            for k in range(n_flat):
                if k + 2 < n_flat:
                    dmaA(*flat[k + 2][:4], sts[k + 2])
                groups = stageA(*flat[k][:4], sts[k])
                pend = []
                if k >= 1:
                    pend += stageB(*flat[k - 1][:4], sts[k - 1])
                if k >= 2:
                    pend += stageC(*flat[k - 2][:4], sts[k - 2])
                for g in groups:
                    g()
                for f in pend:
                    f()
                if k >= 2 and flat[k - 2][1] == n_ch - 1:
                    obj_mlp(flat[k - 2][0])
            for fill in stageB(*flat[-1][:4], sts[-1]):
                fill()
            for fill in stageC(*flat[-2][:4], sts[-2]):
                fill()
            for fill in stageC(*flat[-1][:4], sts[-1]):
                fill()
            obj_mlp(flat[-1][0])

    nc.compile()
    return nc


def _prep_inputs(objects, sender_relations, receiver_relations, relation_info,
                 external_effect_info, rw1, rb1, rw2, rb2, rw3, rb3, rw4, rb4,
                 ow1, ob1, ow2, ob2, ow3, ob3):
    bf16 = ml_dtypes.bfloat16
    f32 = np.float32

    def a(x):
        return np.ascontiguousarray(np.asarray(x, dtype=f32))

    objects = a(objects); sender_relations = a(sender_relations)
    receiver_relations = a(receiver_relations); relation_info = a(relation_info)
    external_effect_info = a(external_effect_info)
    rw1, rb1, rw2, rb2, rw3, rb3, rw4, rb4 = map(a, (rw1, rb1, rw2, rb2, rw3, rb3, rw4, rb4))
    ow1, ob1, ow2, ob2, ow3, ob3 = map(a, (ow1, ob1, ow2, ob2, ow3, ob3))

    info_t_bf = np.ascontiguousarray(relation_info.transpose(0, 2, 1)).astype(bf16)
    s_bf = sender_relations.astype(bf16)
    r_bf = receiver_relations.astype(bf16)
    # [b, rel, obj] -> [b, rel%128, rel//128, obj] so each partition's
    # per-chunk DMA read is contiguous
    r_rel_t = np.ascontiguousarray(
        receiver_relations.transpose(0, 2, 1)
        .reshape(B, N_REL // 128, 128, N_OBJ)
        .transpose(0, 2, 1, 3)).astype(bf16)

    def fold2(w, out_dim):
        # [256, out] -> [128, 2, out] -> [128, 2*out]
        return np.ascontiguousarray(
            w.reshape(2, 128, out_dim).transpose(1, 0, 2).reshape(128, 2 * out_dim))

    wb = np.zeros((128, BF_COLS), dtype=f32)
    o, n = _BF["rw1i"]; wb[:64, o:o + n] = rw1[256:320]
    o, n = _BF["rw2"]; wb[:, o:o + n] = fold2(rw2, HID)
    o, n = _BF["rw3"]; wb[:, o:o + n] = fold2(rw3, HID)
    o, n = _BF["rw4"]; wb[:, o:o + n] = fold2(rw4, EFF_D)
    o, n = _BF["ow1e"]; wb[:, o:o + n] = np.ascontiguousarray(
        ow1[160:288].reshape(128, 2, 128).reshape(128, 256))
    o, n = _BF["ow2"]; wb[:, o:o + n] = fold2(ow2, HID)
    o, n = _BF["ow3"]; wb[:, o:o + n] = fold2(ow3, OUT_D)
    o, n = _BF["B4"]; wb[:, o:o + n] = np.broadcast_to(np.tile(rb4, 4)[None, :], (128, 512))
    wb = wb.astype(bf16)

    wf = np.zeros((128, F32_COLS), dtype=f32)
    o, n = _F32["rb1"]; wf[:, o:o + n] = rb1.reshape(2, 128).T
    o, n = _F32["rb2"]; wf[:, o:o + n] = rb2.reshape(2, 128).T
    o, n = _F32["rb3"]; wf[:, o:o + n] = rb3.reshape(2, 128).T
    o, n = _F32["ob2"]; wf[:, o:o + n] = ob2.reshape(2, 128).T
    o, n = _F32["ob3r"]; wf[:, o:o + n] = np.broadcast_to(ob3[None, :], (128, OUT_D))

    in_maps = []
    for c in range(N_CORES):
        sl = slice(c * B_CORE, (c + 1) * B_CORE)
        m = {
            "wblob_bf": wb,
            "wblob_f32": wf,
            "s_rel": s_bf[sl],
            "r_rel": r_bf[sl],
            "info_t": info_t_bf[sl],
            "r_rel_t": r_rel_t[sl],
        }
        pbl = np.zeros((128, PB_COLS), dtype=f32)
        pf = np.zeros((128, PF_COLS), dtype=f32)
        for bi in range(B_CORE):
            O = objects[c * B_CORE + bi]
            X = external_effect_info[c * B_CORE + bi]
            As = O @ rw1[0:128]
            Arr = O @ rw1[128:256]
            o, n = _PB[f"As{bi}"]; pbl[:, o:o + n] = As
            o, n = _PB[f"Ar{bi}"]; pbl[:, o:o + n] = Arr
            g1 = ow1[0:128].T @ O.T + ow1[128:160].T @ X.T + ob1[:, None]
            o, n = _PF[f"G1pre{bi}"]
            pf[:, o:o + n] = np.ascontiguousarray(
                g1.reshape(2, 128, 128).transpose(1, 0, 2).reshape(128, 256))
        m["pblob_bf"] = pbl.astype(bf16)
        m["pblob_f32"] = pf
        in_maps.append(m)
    return in_maps


def run(in_maps, **spmd_kwargs):
    from concourse.bass_utils import run_bass_kernel_spmd

    if "nc" not in _CACHE:
        _CACHE["nc"] = build_kernel()
    return run_bass_kernel_spmd(_CACHE["nc"], in_maps,
                                core_ids=list(range(N_CORES)), **spmd_kwargs)


def kernel(**inputs) -> np.ndarray:
    in_maps = _prep_inputs(**inputs)
    res = run(in_maps)
    out = np.concatenate([r["out"].reshape(-1, OUT_D) for r in res.results], axis=0)
    return np.ascontiguousarray(out, dtype=np.float32)


# revision 23
# speedup vs baseline: 1.1701x; 1.1701x over previous
"""Trainium2 Bass kernel for BasicInteractionNetworkModule.

Data-parallel over batch (B=16) across 8 NeuronCores, 2 batches/core.

Math (per batch b):
  senders   = S^T @ O          [R, 128]   (S = sender_relations [128, R])
  receivers = R_rel^T @ O      [R, 128]
  rel_x = [senders, receivers, info]   [R, 320]
  h = relu-MLP(rel_x): 320 -> 256 -> 256 -> 256 -> 128 (relu after every layer)
  eff_recv = R_rel @ effects   [128, 128]
  obj_x = [O, ext, eff_recv]   [128, 288]
  out = relu-MLP2(obj_x): 288 -> 256 -> 256 -> 128 (no final relu)

Device strategy: relation-MLP activations feature-major (partition = feature)
so every layer is out^T = W^T @ H^T with the moving operand streaming 512-col
slabs at 1 col/cycle, all in bf16. Layer-1 folds the sender/receiver
projections via host-precomputed A_s = O @ rw1[:128], A_r = O @ rw1[128:256].
Three-stage software pipeline per chunk k: A(k)=L1/L2/L3, B(k-1)=L4,
C(k-2)=aggregation, so every PSUM-evac -> stationary-reload handoff has a
full chunk of slack. Each 512-col PSUM slab is one bank (psB rotates 5).
L4's bias is folded into the evacuation (DVE add, ACT relu). The aggregation
accumulates all 127 rel-blocks of a batch into a persistent PSUM region.
The object MLP's objects/ext contributions are host-precomputed (G1pre).
"""

import numpy as np
import ml_dtypes

B, N_OBJ, N_REL = 16, 128, 16256
OBJ_D, REL_D, EFF_D, EXT_D, OUT_D = 128, 64, 128, 32, 128
HID = 256
N_CORES = 8
B_CORE = B // N_CORES  # 2
M_CHUNK = 1024

_CACHE = {}

# column offsets in the shared bf16 weight blob
_BF = {}
_off = 0
for _name, _n in [("rw1i", 256), ("rw2", 512), ("rw3", 512), ("rw4", 256),
                  ("ow1e", 256), ("ow2", 512), ("ow3", 256), ("B4", 512)]:
    _BF[_name] = (_off, _n)
    _off += _n
BF_COLS = _off
# per-core bf16 blob: As/Ar per batch
_PB = {}
_off = 0
for _name, _n in [("As0", 256), ("Ar0", 256), ("As1", 256), ("Ar1", 256)]:
    _PB[_name] = (_off, _n)
    _off += _n
PB_COLS = _off
# shared f32 blob
_F32 = {}
_off = 0
for _name, _n in [("rb1", 2), ("rb2", 2), ("rb3", 2), ("ob2", 2), ("ob3r", 128)]:
    _F32[_name] = (_off, _n)
    _off += _n
F32_COLS = _off
# per-core f32 blob: G1pre per batch [128, 2, 128]
_PF = {"G1pre0": (0, 256), "G1pre1": (256, 256)}
PF_COLS = 512


def _chunks():
    out = []
    base = 0
    while base < N_REL:
        mc = min(M_CHUNK, N_REL - base)
        out.append((base, mc))
        base += mc
    return out


def _mtiles(mc):
    out = []
    base = 0
    while base < mc:
        n = min(512, mc - base)
        out.append((base, n))
        base += n
    return out


def build_kernel():
    from concourse import bacc
    import concourse.mybir as mybir
    import concourse.tile as tile

    F32 = mybir.dt.float32
    BF16 = mybir.dt.bfloat16
    RELU = mybir.ActivationFunctionType.Relu
    ADD = mybir.AluOpType.add
    MAX = mybir.AluOpType.max
    MULT = mybir.AluOpType.mult

    nc = bacc.Bacc(None)

    S_d = nc.dram_tensor("s_rel", [B_CORE, N_OBJ, N_REL], BF16, kind="ExternalInput")
    R_d = nc.dram_tensor("r_rel", [B_CORE, N_OBJ, N_REL], BF16, kind="ExternalInput")
    IT_d = nc.dram_tensor("info_t", [B_CORE, REL_D, N_REL], BF16, kind="ExternalInput")
    # receiver_relations pre-permuted on host: [b, p, s, o] = R^T[b, s*128+p, o]
    # so each partition's per-chunk read is one contiguous run
    RT_d = nc.dram_tensor("r_rel_t", [B_CORE, 128, N_REL // 128, N_OBJ], BF16,
                          kind="ExternalInput")

    WB_d = nc.dram_tensor("wblob_bf", [128, BF_COLS], BF16, kind="ExternalInput")
    PB_d = nc.dram_tensor("pblob_bf", [128, PB_COLS], BF16, kind="ExternalInput")
    WF_d = nc.dram_tensor("wblob_f32", [128, F32_COLS], F32, kind="ExternalInput")
    PF_d = nc.dram_tensor("pblob_f32", [128, PF_COLS], F32, kind="ExternalInput")

    out_d = nc.dram_tensor("out", [B_CORE, N_OBJ, OUT_D], F32, kind="ExternalOutput")

    with tile.TileContext(nc) as tc:
        with (
            tc.tile_pool(name="wts", bufs=1) as wts,
            tc.tile_pool(name="perb", bufs=2) as perb,
            tc.tile_pool(name="cin", bufs=6) as cin,
            tc.tile_pool(name="acts", bufs=3) as acts,
            tc.tile_pool(name="psB", bufs=7, space="PSUM") as psB,
            tc.tile_pool(name="psE", bufs=1, space="PSUM") as psE,
        ):
            WB = wts.tile([128, BF_COLS], BF16)
            PBt = wts.tile([128, PB_COLS], BF16)
            WF = wts.tile([128, F32_COLS], F32)
            PF = wts.tile([128, PF_COLS], F32)

            def bf(name):
                o, n = _BF[name]
                return WB[:, o:o + n]

            def pb(name):
                o, n = _PB[name]
                return PBt[:, o:o + n]

            def f32(name):
                o, n = _F32[name]
                return WF[:, o:o + n]

            _o, _n = _BF["rw1i"]
            rw1i = WB[:64, _o:_o + _n].rearrange("p (two h) -> p two h", two=2)
            rw2 = bf("rw2").rearrange("p (two h) -> p two h", two=2)
            rw3 = bf("rw3").rearrange("p (two h) -> p two h", two=2)
            rw4 = bf("rw4").rearrange("p (two h) -> p two h", two=2)
            ow1e = bf("ow1e").rearrange("p (two h) -> p two h", two=2)
            ow2 = bf("ow2").rearrange("p (two h) -> p two h", two=2)
            ow3 = bf("ow3").rearrange("p (two h) -> p two h", two=2)
            B4 = bf("B4")
            rb1 = f32("rb1")
            rb2 = f32("rb2")
            rb3 = f32("rb3")
            ob2 = f32("ob2")
            ob3r = f32("ob3r")
            As = [pb("As0"), pb("As1")]
            Ar = [pb("Ar0"), pb("Ar1")]
            G1pre = [PF[:, 0:256].rearrange("p (two h) -> p two h", two=2),
                     PF[:, 256:512].rearrange("p (two h) -> p two h", two=2)]

            chs = _chunks()
            n_ch = len(chs)
            effaccs = {}
            effacc_all = psE.tile([128, B_CORE * N_OBJ], F32, tag="effacc")
            for b in range(B_CORE):
                effaccs[b] = effacc_all[:, b * N_OBJ:(b + 1) * N_OBJ]

            def dmaA(b, ci, base, mc, st):
                ns = mc // 128
                S_c = cin.tile([N_OBJ, M_CHUNK], BF16, tag="S_c")
                R_c = cin.tile([N_OBJ, M_CHUNK], BF16, tag="R_c")
                I_c = cin.tile([REL_D, M_CHUNK], BF16, tag="I_c")
                RT_c = cin.tile([128, M_CHUNK // 128, N_OBJ], BF16, tag="RT_c")
                nc.sync.dma_start(S_c[:, :mc], S_d[b, :, base:base + mc])
                nc.sync.dma_start(I_c[:, :mc], IT_d[b, :, base:base + mc])
                nc.gpsimd.dma_start(R_c[:, :mc], R_d[b, :, base:base + mc])
                s0 = base // 128
                nc.gpsimd.dma_start(RT_c[:, :ns, :], RT_d[b, :, s0:s0 + ns, :])
                st['S_c'] = S_c; st['R_c'] = R_c; st['I_c'] = I_c; st['RT_c'] = RT_c

            def stageA(b, ci, base, mc, st):
                """Returns a list of slab-group closures (stall-prone)."""
                S_c = st['S_c']; R_c = st['R_c']; I_c = st['I_c']
                H1 = acts.tile([128, 2, M_CHUNK], BF16, tag="H1")
                H2 = acts.tile([128, 2, M_CHUNK], BF16, tag="H2")
                H3 = acts.tile([128, 2, M_CHUNK], BF16, tag="H3")
                st['H3'] = H3
                groups = []

                h0, h1 = slice(0, 128), slice(128, 256)

                def l1(sl, n):
                    # both halves' accumulation chains interleaved over two
                    # psum banks: a bank gets a full stream-time between its
                    # chained read-modify-writes, hiding the write drain
                    def go():
                        pa = psB.tile([128, 512], F32, tag="ps")
                        pb2 = psB.tile([128, 512], F32, tag="ps")
                        nc.tensor.matmul(pa[:, :n], rw1i[:, 0, :], I_c[:, sl], start=True, stop=False)
                        nc.tensor.matmul(pb2[:, :n], rw1i[:, 1, :], I_c[:, sl], start=True, stop=False)
                        nc.tensor.matmul(pa[:, :n], As[b][:, h0], S_c[:, sl], start=False, stop=False)
                        nc.tensor.matmul(pb2[:, :n], As[b][:, h1], S_c[:, sl], start=False, stop=False)
                        nc.tensor.matmul(pa[:, :n], Ar[b][:, h0], R_c[:, sl], start=False, stop=True)
                        nc.tensor.matmul(pb2[:, :n], Ar[b][:, h1], R_c[:, sl], start=False, stop=True)
                        nc.scalar.activation(H1[:, 0, sl], pa[:, :n], RELU,
                                             bias=rb1[:, 0:1], scale=1.0)
                        nc.scalar.activation(H1[:, 1, sl], pb2[:, :n], RELU,
                                             bias=rb1[:, 1:2], scale=1.0)
                    return go

                def l23(sl, n, w, Hin, Hout, rb):
                    def go():
                        pa = psB.tile([128, 512], F32, tag="ps")
                        pb2 = psB.tile([128, 512], F32, tag="ps")
                        nc.tensor.matmul(pa[:, :n], w[:, 0, h0], Hin[:, 0, sl], start=True, stop=False)
                        nc.tensor.matmul(pb2[:, :n], w[:, 0, h1], Hin[:, 0, sl], start=True, stop=False)
                        nc.tensor.matmul(pa[:, :n], w[:, 1, h0], Hin[:, 1, sl], start=False, stop=True)
                        nc.tensor.matmul(pb2[:, :n], w[:, 1, h1], Hin[:, 1, sl], start=False, stop=True)
                        nc.scalar.activation(Hout[:, 0, sl], pa[:, :n], RELU,
                                             bias=rb[:, 0:1], scale=1.0)
                        nc.vector.tensor_scalar(Hout[:, 1, sl], pb2[:, :n],
                                                rb[:, 1:2], 0.0, ADD, MAX)
                    return go

                for mt, n in _mtiles(mc):
                    groups.append(l1(slice(mt, mt + n), n))
                for mt, n in _mtiles(mc):
                    groups.append(l23(slice(mt, mt + n), n, rw2, H1, H2, rb2))
                for mt, n in _mtiles(mc):
                    groups.append(l23(slice(mt, mt + n), n, rw3, H2, H3, rb3))
                return groups

            def stageB(b, ci, base, mc, st):
                """Returns closures of slack-rich L4 work (never stalls)."""
                ns = mc // 128
                H3 = st['H3']
                E3t = acts.tile([128, M_CHUNK], BF16, tag="E3t")
                E3 = acts.tile([128, M_CHUNK], BF16, tag="E3")
                st['E3'] = E3
                fills = []
                for g in range(0, ns, 4):
                    ge = min(g + 4, ns)

                    def l4(g, ge):
                        def go():
                            span = (ge - g) * 128
                            gsl = slice(g * 128, g * 128 + span)
                            ps4 = psB.tile([128, 512], F32, tag="ps")
                            for sj in range(g, ge):
                                sl = slice(sj * 128, (sj + 1) * 128)
                                psl = slice((sj - g) * 128, (sj - g + 1) * 128)
                                nc.tensor.matmul(ps4[:, psl], H3[:, 0, sl], rw4[:, 0, :],
                                                 start=True, stop=False, skip_group_check=True)
                                nc.tensor.matmul(ps4[:, psl], H3[:, 1, sl], rw4[:, 1, :],
                                                 start=False, stop=True, skip_group_check=True)
                            nc.vector.scalar_tensor_tensor(E3t[:, gsl], ps4[:, :span], 1.0,
                                                           B4[:, :span], MULT, ADD)
                            nc.vector.tensor_scalar_max(E3[:, gsl], E3t[:, gsl], 0.0)
                        return go

                    fills.append(l4(g, ge))
                return fills

            def stageC(b, ci, base, mc, st):
                """Returns closures of slack-rich aggregation work."""
                ns = mc // 128
                E3 = st['E3']; RT_c = st['RT_c']
                effacc = effaccs[b]

                def agg(s0, s1):
                    def go():
                        for sj in range(s0, s1):
                            nc.tensor.matmul(effacc, E3[:, sj * 128:(sj + 1) * 128],
                                             RT_c[:, sj, :],
                                             start=(ci == 0 and sj == 0),
                                             stop=(ci == n_ch - 1 and sj == ns - 1),
                                             skip_group_check=True)
                    return go

                mid = (ns + 1) // 2
                return [agg(0, mid), agg(mid, ns)]

            def obj_mlp(b):
                effacc = effaccs[b]
                effTb = perb.tile([128, N_OBJ], BF16, tag="effTb")
                nc.scalar.copy(effTb, effacc)
                G1 = perb.tile([128, 2, N_OBJ], BF16, tag="G1")
                G2 = perb.tile([128, 2, N_OBJ], BF16, tag="G2")
                Gt = perb.tile([128, N_OBJ], F32, tag="Gt")
                for p2 in range(2):
                    ps = psB.tile([128, 512], F32, tag="ps")
                    nc.tensor.matmul(ps[:, :N_OBJ], ow1e[:, p2, :], effTb, start=True, stop=True)
                    nc.vector.scalar_tensor_tensor(Gt, ps[:, :N_OBJ], 1.0,
                                                   G1pre[b][:, p2, :], MULT, ADD)
                    nc.scalar.activation(G1[:, p2, :], Gt, RELU)
                for p2 in range(2):
                    ps = psB.tile([128, 512], F32, tag="ps")
                    h = slice(p2 * 128, (p2 + 1) * 128)
                    nc.tensor.matmul(ps[:, :N_OBJ], ow2[:, 0, h], G1[:, 0, :], start=True, stop=False)
                    nc.tensor.matmul(ps[:, :N_OBJ], ow2[:, 1, h], G1[:, 1, :], start=False, stop=True)
                    nc.scalar.activation(G2[:, p2, :], ps[:, :N_OBJ], RELU,
                                         bias=ob2[:, p2:p2 + 1], scale=1.0)
                ps = psB.tile([128, 512], F32, tag="ps")
                nc.tensor.matmul(ps[:, :OUT_D], G2[:, 0, :], ow3[:, 0, :], start=True, stop=False)
                nc.tensor.matmul(ps[:, :OUT_D], G2[:, 1, :], ow3[:, 1, :], start=False, stop=True)
                ob = perb.tile([N_OBJ, OUT_D], F32, tag="ob")
                nc.vector.tensor_tensor(ob, ps[:, :OUT_D], ob3r, ADD)
                nc.sync.dma_start(out_d[b], ob)

            flat = [(b, ci, base, mc) for b in range(B_CORE)
                    for ci, (base, mc) in enumerate(chs)]
            n_flat = len(flat)
            sts = [dict() for _ in flat]

            # critical small weights first, then chunk-0/1 streams, then the
            # bulk weights split across both rings; loop prefetches 2 chunks
            nc.sync.dma_start(WF, WF_d[:])
            nc.sync.dma_start(WB[:, 0:256], WB_d[:, 0:256])        # rw1i
            nc.gpsimd.dma_start(PBt, PB_d[:])                      # As/Ar
            dmaA(*flat[0][:4], sts[0])
            nc.sync.dma_start(WB[:, 256:1280], WB_d[:, 256:1280])  # rw2,rw3
            dmaA(*flat[1][:4], sts[1])
            nc.gpsimd.dma_start(WB[:, 1280:BF_COLS], WB_d[:, 1280:BF_COLS])
            nc.gpsimd.dma_start(PF, PF_d[:])

            for k in range(n_flat):
                if k + 2 < n_flat:
                    dmaA(*flat[k + 2][:4], sts[k + 2])
                groups = stageA(*flat[k][:4], sts[k])
                pend = []
                if k >= 1:
                    pend += stageB(*flat[k - 1][:4], sts[k - 1])
                if k >= 2:
                    pend += stageC(*flat[k - 2][:4], sts[k - 2])
                for g in groups:
                    g()
                for f in pend:
                    f()
                if k >= 2 and flat[k - 2][1] == n_ch - 1:
                    obj_mlp(flat[k - 2][0])
            for fill in stageB(*flat[-1][:4], sts[-1]):
                fill()
            for fill in stageC(*flat[-2][:4], sts[-2]):
                fill()
            for fill in stageC(*flat[-1][:4], sts[-1]):
                fill()
            obj_mlp(flat[-1][0])

    nc.compile()
    return nc


def _prep_inputs(objects, sender_relations, receiver_relations, relation_info,
                 external_effect_info, rw1, rb1, rw2, rb2, rw3, rb3, rw4, rb4,
                 ow1, ob1, ow2, ob2, ow3, ob3):
    bf16 = ml_dtypes.bfloat16
    f32 = np.float32

    def a(x):
        return np.ascontiguousarray(np.asarray(x, dtype=f32))

    objects = a(objects); sender_relations = a(sender_relations)
    receiver_relations = a(receiver_relations); relation_info = a(relation_info)
    external_effect_info = a(external_effect_info)
    rw1, rb1, rw2, rb2, rw3, rb3, rw4, rb4 = map(a, (rw1, rb1, rw2, rb2, rw3, rb3, rw4, rb4))
    ow1, ob1, ow2, ob2, ow3, ob3 = map(a, (ow1, ob1, ow2, ob2, ow3, ob3))

    info_t_bf = np.ascontiguousarray(relation_info.transpose(0, 2, 1)).astype(bf16)
    s_bf = sender_relations.astype(bf16)
    r_bf = receiver_relations.astype(bf16)
    # [b, rel, obj] -> [b, rel%128, rel//128, obj] so each partition's
    # per-chunk DMA read is contiguous
    r_rel_t = np.ascontiguousarray(
        receiver_relations.transpose(0, 2, 1)
        .reshape(B, N_REL // 128, 128, N_OBJ)
        .transpose(0, 2, 1, 3)).astype(bf16)

    def fold2(w, out_dim):
        # [256, out] -> [128, 2, out] -> [128, 2*out]
        return np.ascontiguousarray(
            w.reshape(2, 128, out_dim).transpose(1, 0, 2).reshape(128, 2 * out_dim))

    wb = np.zeros((128, BF_COLS), dtype=f32)
    o, n = _BF["rw1i"]; wb[:64, o:o + n] = rw1[256:320]
    o, n = _BF["rw2"]; wb[:, o:o + n] = fold2(rw2, HID)
    o, n = _BF["rw3"]; wb[:, o:o + n] = fold2(rw3, HID)
    o, n = _BF["rw4"]; wb[:, o:o + n] = fold2(rw4, EFF_D)
    o, n = _BF["ow1e"]; wb[:, o:o + n] = np.ascontiguousarray(
        ow1[160:288].reshape(128, 2, 128).reshape(128, 256))
    o, n = _BF["ow2"]; wb[:, o:o + n] = fold2(ow2, HID)
    o, n = _BF["ow3"]; wb[:, o:o + n] = fold2(ow3, OUT_D)
    o, n = _BF["B4"]; wb[:, o:o + n] = np.broadcast_to(np.tile(rb4, 4)[None, :], (128, 512))
    wb = wb.astype(bf16)

    wf = np.zeros((128, F32_COLS), dtype=f32)
    o, n = _F32["rb1"]; wf[:, o:o + n] = rb1.reshape(2, 128).T
    o, n = _F32["rb2"]; wf[:, o:o + n] = rb2.reshape(2, 128).T
    o, n = _F32["rb3"]; wf[:, o:o + n] = rb3.reshape(2, 128).T
    o, n = _F32["ob2"]; wf[:, o:o + n] = ob2.reshape(2, 128).T
    o, n = _F32["ob3r"]; wf[:, o:o + n] = np.broadcast_to(ob3[None, :], (128, OUT_D))

    in_maps = []
    for c in range(N_CORES):
        sl = slice(c * B_CORE, (c + 1) * B_CORE)
        m = {
            "wblob_bf": wb,
            "wblob_f32": wf,
            "s_rel": s_bf[sl],
            "r_rel": r_bf[sl],
            "info_t": info_t_bf[sl],
            "r_rel_t": r_rel_t[sl],
        }
        pbl = np.zeros((128, PB_COLS), dtype=f32)
        pf = np.zeros((128, PF_COLS), dtype=f32)
        for bi in range(B_CORE):
            O = objects[c * B_CORE + bi]
            X = external_effect_info[c * B_CORE + bi]
            As = O @ rw1[0:128]
            Arr = O @ rw1[128:256]
            o, n = _PB[f"As{bi}"]; pbl[:, o:o + n] = As
            o, n = _PB[f"Ar{bi}"]; pbl[:, o:o + n] = Arr
            g1 = ow1[0:128].T @ O.T + ow1[128:160].T @ X.T + ob1[:, None]
            o, n = _PF[f"G1pre{bi}"]
            pf[:, o:o + n] = np.ascontiguousarray(
                g1.reshape(2, 128, 128).transpose(1, 0, 2).reshape(128, 256))
        m["pblob_bf"] = pbl.astype(bf16)
        m["pblob_f32"] = pf
        in_maps.append(m)
    return in_maps


def run(in_maps, **spmd_kwargs):
    from concourse.bass_utils import run_bass_kernel_spmd

    if "nc" not in _CACHE:
        _CACHE["nc"] = build_kernel()
    return run_bass_kernel_spmd(_CACHE["nc"], in_maps,
                                core_ids=list(range(N_CORES)), **spmd_kwargs)


def kernel(**inputs) -> np.ndarray:
    in_maps = _prep_inputs(**inputs)
    res = run(in_maps)
    out = np.concatenate([r["out"].reshape(-1, OUT_D) for r in res.results], axis=0)
    return np.ascontiguousarray(out, dtype=np.float32)


# revision 24
# speedup vs baseline: 1.1847x; 1.0125x over previous
"""Trainium2 Bass kernel for BasicInteractionNetworkModule.

Data-parallel over batch (B=16) across 8 NeuronCores, 2 batches/core.

Math (per batch b):
  senders   = S^T @ O          [R, 128]   (S = sender_relations [128, R])
  receivers = R_rel^T @ O      [R, 128]
  rel_x = [senders, receivers, info]   [R, 320]
  h = relu-MLP(rel_x): 320 -> 256 -> 256 -> 256 -> 128 (relu after every layer)
  eff_recv = R_rel @ effects   [128, 128]
  obj_x = [O, ext, eff_recv]   [128, 288]
  out = relu-MLP2(obj_x): 288 -> 256 -> 256 -> 128 (no final relu)

Device strategy: relation-MLP activations feature-major (partition = feature)
so every layer is out^T = W^T @ H^T with the moving operand streaming 512-col
slabs at 1 col/cycle, all in bf16. Layer-1 folds the sender/receiver
projections via host-precomputed A_s = O @ rw1[:128], A_r = O @ rw1[128:256].
Three-stage software pipeline per chunk k: A(k)=L1/L2/L3, B(k-1)=L4,
C(k-2)=aggregation, so every PSUM-evac -> stationary-reload handoff has a
full chunk of slack. Each 512-col PSUM slab is one bank (psB rotates 5).
L4's bias is folded into the evacuation (DVE add, ACT relu). The aggregation
accumulates all 127 rel-blocks of a batch into a persistent PSUM region.
The object MLP's objects/ext contributions are host-precomputed (G1pre).
"""

import numpy as np
import ml_dtypes

B, N_OBJ, N_REL = 16, 128, 16256
OBJ_D, REL_D, EFF_D, EXT_D, OUT_D = 128, 64, 128, 32, 128
HID = 256
N_CORES = 8
B_CORE = B // N_CORES  # 2
M_CHUNK = 1024

_CACHE = {}

# column offsets in the shared bf16 weight blob
_BF = {}
_off = 0
for _name, _n in [("rw1i", 256), ("rw2", 512), ("rw3", 512), ("rw4", 256),
                  ("ow1e", 256), ("ow2", 512), ("ow3", 256), ("B4", 512)]:
    _BF[_name] = (_off, _n)
    _off += _n
BF_COLS = _off
# per-core bf16 blob: As/Ar per batch
_PB = {}
_off = 0
for _name, _n in [("As0", 256), ("Ar0", 256), ("As1", 256), ("Ar1", 256)]:
    _PB[_name] = (_off, _n)
    _off += _n
PB_COLS = _off
# shared f32 blob
_F32 = {}
_off = 0
for _name, _n in [("rb1", 2), ("rb2", 2), ("rb3", 2), ("ob2", 2), ("ob3r", 128)]:
    _F32[_name] = (_off, _n)
    _off += _n
F32_COLS = _off
# per-core f32 blob: G1pre per batch [128, 2, 128]
_PF = {"G1pre0": (0, 256), "G1pre1": (256, 256)}
PF_COLS = 512


def _chunks():
    out = []
    base = 0
    while base < N_REL:
        mc = min(M_CHUNK, N_REL - base)
        out.append((base, mc))
        base += mc
    return out


def _mtiles(mc):
    out = []
    base = 0
    while base < mc:
        n = min(512, mc - base)
        out.append((base, n))
        base += n
    return out


def build_kernel():
    from concourse import bacc
    import concourse.mybir as mybir
    import concourse.tile as tile

    F32 = mybir.dt.float32
    BF16 = mybir.dt.bfloat16
    RELU = mybir.ActivationFunctionType.Relu
    ADD = mybir.AluOpType.add
    MAX = mybir.AluOpType.max
    MULT = mybir.AluOpType.mult

    nc = bacc.Bacc(None)

    S_d = nc.dram_tensor("s_rel", [B_CORE, N_OBJ, N_REL], BF16, kind="ExternalInput")
    R_d = nc.dram_tensor("r_rel", [B_CORE, N_OBJ, N_REL], BF16, kind="ExternalInput")
    IT_d = nc.dram_tensor("info_t", [B_CORE, REL_D, N_REL], BF16, kind="ExternalInput")
    # receiver_relations pre-permuted on host: [b, p, s, o] = R^T[b, s*128+p, o]
    # so each partition's per-chunk read is one contiguous run
    RT_d = nc.dram_tensor("r_rel_t", [B_CORE, 128, N_REL // 128, N_OBJ], BF16,
                          kind="ExternalInput")

    WB_d = nc.dram_tensor("wblob_bf", [128, BF_COLS], BF16, kind="ExternalInput")
    PB_d = nc.dram_tensor("pblob_bf", [128, PB_COLS], BF16, kind="ExternalInput")
    WF_d = nc.dram_tensor("wblob_f32", [128, F32_COLS], F32, kind="ExternalInput")
    PF_d = nc.dram_tensor("pblob_f32", [128, PF_COLS], F32, kind="ExternalInput")

    out_d = nc.dram_tensor("out", [B_CORE, N_OBJ, OUT_D], F32, kind="ExternalOutput")

    with tile.TileContext(nc) as tc:
        with (
            tc.tile_pool(name="wts", bufs=1) as wts,
            tc.tile_pool(name="perb", bufs=2) as perb,
            tc.tile_pool(name="cin", bufs=6) as cin,
            tc.tile_pool(name="acts", bufs=3) as acts,
            tc.tile_pool(name="psB", bufs=5, space="PSUM") as psB,
            tc.tile_pool(name="ps4", bufs=2, space="PSUM") as ps4p,
            tc.tile_pool(name="psE", bufs=1, space="PSUM") as psE,
        ):
            WB = wts.tile([128, BF_COLS], BF16)
            PBt = wts.tile([128, PB_COLS], BF16)
            WF = wts.tile([128, F32_COLS], F32)
            PF = wts.tile([128, PF_COLS], F32)

            def bf(name):
                o, n = _BF[name]
                return WB[:, o:o + n]

            def pb(name):
                o, n = _PB[name]
                return PBt[:, o:o + n]

            def f32(name):
                o, n = _F32[name]
                return WF[:, o:o + n]

            _o, _n = _BF["rw1i"]
            rw1i = WB[:64, _o:_o + _n].rearrange("p (two h) -> p two h", two=2)
            rw2 = bf("rw2").rearrange("p (two h) -> p two h", two=2)
            rw3 = bf("rw3").rearrange("p (two h) -> p two h", two=2)
            rw4 = bf("rw4").rearrange("p (two h) -> p two h", two=2)
            ow1e = bf("ow1e").rearrange("p (two h) -> p two h", two=2)
            ow2 = bf("ow2").rearrange("p (two h) -> p two h", two=2)
            ow3 = bf("ow3").rearrange("p (two h) -> p two h", two=2)
            B4 = bf("B4")
            rb1 = f32("rb1")
            rb2 = f32("rb2")
            rb3 = f32("rb3")
            ob2 = f32("ob2")
            ob3r = f32("ob3r")
            As = [pb("As0"), pb("As1")]
            Ar = [pb("Ar0"), pb("Ar1")]
            G1pre = [PF[:, 0:256].rearrange("p (two h) -> p two h", two=2),
                     PF[:, 256:512].rearrange("p (two h) -> p two h", two=2)]

            chs = _chunks()
            n_ch = len(chs)
            effaccs = {}
            effacc_all = psE.tile([128, B_CORE * N_OBJ], F32, tag="effacc")
            for b in range(B_CORE):
                effaccs[b] = effacc_all[:, b * N_OBJ:(b + 1) * N_OBJ]

            def dmaA(b, ci, base, mc, st):
                ns = mc // 128
                S_c = cin.tile([N_OBJ, M_CHUNK], BF16, tag="S_c")
                R_c = cin.tile([N_OBJ, M_CHUNK], BF16, tag="R_c")
                I_c = cin.tile([REL_D, M_CHUNK], BF16, tag="I_c")
                RT_c = cin.tile([128, M_CHUNK // 128, N_OBJ], BF16, tag="RT_c")
                nc.sync.dma_start(S_c[:, :mc], S_d[b, :, base:base + mc])
                nc.sync.dma_start(I_c[:, :mc], IT_d[b, :, base:base + mc])
                nc.gpsimd.dma_start(R_c[:, :mc], R_d[b, :, base:base + mc])
                s0 = base // 128
                nc.gpsimd.dma_start(RT_c[:, :ns, :], RT_d[b, :, s0:s0 + ns, :])
                st['S_c'] = S_c; st['R_c'] = R_c; st['I_c'] = I_c; st['RT_c'] = RT_c

            def stageA(b, ci, base, mc, st):
                """Returns a list of slab-group closures (stall-prone)."""
                S_c = st['S_c']; R_c = st['R_c']; I_c = st['I_c']
                H1 = acts.tile([128, 2, M_CHUNK], BF16, tag="H1")
                H2 = acts.tile([128, 2, M_CHUNK], BF16, tag="H2")
                H3 = acts.tile([128, 2, M_CHUNK], BF16, tag="H3")
                st['H3'] = H3
                groups = []

                h0, h1 = slice(0, 128), slice(128, 256)

                def l1(sl, n):
                    # both halves' accumulation chains interleaved over two
                    # psum banks: a bank gets a full stream-time between its
                    # chained read-modify-writes, hiding the write drain
                    def go():
                        pa = psB.tile([128, 512], F32, tag="ps")
                        pb2 = psB.tile([128, 512], F32, tag="ps")
                        nc.tensor.matmul(pa[:, :n], rw1i[:, 0, :], I_c[:, sl], start=True, stop=False)
                        nc.tensor.matmul(pb2[:, :n], rw1i[:, 1, :], I_c[:, sl], start=True, stop=False)
                        nc.tensor.matmul(pa[:, :n], As[b][:, h0], S_c[:, sl], start=False, stop=False)
                        nc.tensor.matmul(pb2[:, :n], As[b][:, h1], S_c[:, sl], start=False, stop=False)
                        nc.tensor.matmul(pa[:, :n], Ar[b][:, h0], R_c[:, sl], start=False, stop=True)
                        nc.tensor.matmul(pb2[:, :n], Ar[b][:, h1], R_c[:, sl], start=False, stop=True)
                        nc.scalar.activation(H1[:, 0, sl], pa[:, :n], RELU,
                                             bias=rb1[:, 0:1], scale=1.0)
                        nc.scalar.activation(H1[:, 1, sl], pb2[:, :n], RELU,
                                             bias=rb1[:, 1:2], scale=1.0)
                    return go

                def l23(sl, n, w, Hin, Hout, rb):
                    def go():
                        pa = psB.tile([128, 512], F32, tag="ps")
                        pb2 = psB.tile([128, 512], F32, tag="ps")
                        nc.tensor.matmul(pa[:, :n], w[:, 0, h0], Hin[:, 0, sl], start=True, stop=False)
                        nc.tensor.matmul(pb2[:, :n], w[:, 0, h1], Hin[:, 0, sl], start=True, stop=False)
                        nc.tensor.matmul(pa[:, :n], w[:, 1, h0], Hin[:, 1, sl], start=False, stop=True)
                        nc.tensor.matmul(pb2[:, :n], w[:, 1, h1], Hin[:, 1, sl], start=False, stop=True)
                        nc.scalar.activation(Hout[:, 0, sl], pa[:, :n], RELU,
                                             bias=rb[:, 0:1], scale=1.0)
                        nc.vector.tensor_scalar(Hout[:, 1, sl], pb2[:, :n],
                                                rb[:, 1:2], 0.0, ADD, MAX)
                    return go

                for mt, n in _mtiles(mc):
                    groups.append(l1(slice(mt, mt + n), n))
                for mt, n in _mtiles(mc):
                    groups.append(l23(slice(mt, mt + n), n, rw2, H1, H2, rb2))
                for mt, n in _mtiles(mc):
                    groups.append(l23(slice(mt, mt + n), n, rw3, H2, H3, rb3))
                return groups

            def stageB(b, ci, base, mc, st):
                """Returns closures of slack-rich L4 work (never stalls)."""
                ns = mc // 128
                H3 = st['H3']
                E3t = acts.tile([128, M_CHUNK], BF16, tag="E3t")
                E3 = acts.tile([128, M_CHUNK], BF16, tag="E3")
                st['E3'] = E3
                fills = []
                for g in range(0, ns, 4):
                    ge = min(g + 4, ns)

                    def l4(g, ge):
                        def go():
                            span = (ge - g) * 128
                            gsl = slice(g * 128, g * 128 + span)
                            ps4 = ps4p.tile([128, 512], F32, tag="ps4")
                            for sj in range(g, ge):
                                sl = slice(sj * 128, (sj + 1) * 128)
                                psl = slice((sj - g) * 128, (sj - g + 1) * 128)
                                nc.tensor.matmul(ps4[:, psl], H3[:, 0, sl], rw4[:, 0, :],
                                                 start=True, stop=False, skip_group_check=True)
                                nc.tensor.matmul(ps4[:, psl], H3[:, 1, sl], rw4[:, 1, :],
                                                 start=False, stop=True, skip_group_check=True)
                            nc.vector.scalar_tensor_tensor(E3t[:, gsl], ps4[:, :span], 1.0,
                                                           B4[:, :span], MULT, ADD)
                            nc.vector.tensor_scalar_max(E3[:, gsl], E3t[:, gsl], 0.0)
                        return go

                    fills.append(l4(g, ge))
                return fills

            def stageC(b, ci, base, mc, st):
                """Returns closures of slack-rich aggregation work."""
                ns = mc // 128
                E3 = st['E3']; RT_c = st['RT_c']
                effacc = effaccs[b]

                def agg(s0, s1):
                    def go():
                        for sj in range(s0, s1):
                            nc.tensor.matmul(effacc, E3[:, sj * 128:(sj + 1) * 128],
                                             RT_c[:, sj, :],
                                             start=(ci == 0 and sj == 0),
                                             stop=(ci == n_ch - 1 and sj == ns - 1),
                                             skip_group_check=True)
                    return go

                mid = (ns + 1) // 2
                return [agg(0, mid), agg(mid, ns)]

            def obj_mlp(b):
                effacc = effaccs[b]
                effTb = perb.tile([128, N_OBJ], BF16, tag="effTb")
                nc.scalar.copy(effTb, effacc)
                G1 = perb.tile([128, 2, N_OBJ], BF16, tag="G1")
                G2 = perb.tile([128, 2, N_OBJ], BF16, tag="G2")
                Gt = perb.tile([128, N_OBJ], F32, tag="Gt")
                for p2 in range(2):
                    ps = ps4p.tile([128, 512], F32, tag="ps4")
                    nc.tensor.matmul(ps[:, :N_OBJ], ow1e[:, p2, :], effTb, start=True, stop=True)
                    nc.vector.scalar_tensor_tensor(Gt, ps[:, :N_OBJ], 1.0,
                                                   G1pre[b][:, p2, :], MULT, ADD)
                    nc.scalar.activation(G1[:, p2, :], Gt, RELU)
                for p2 in range(2):
                    ps = ps4p.tile([128, 512], F32, tag="ps4")
                    h = slice(p2 * 128, (p2 + 1) * 128)
                    nc.tensor.matmul(ps[:, :N_OBJ], ow2[:, 0, h], G1[:, 0, :], start=True, stop=False)
                    nc.tensor.matmul(ps[:, :N_OBJ], ow2[:, 1, h], G1[:, 1, :], start=False, stop=True)
                    nc.scalar.activation(G2[:, p2, :], ps[:, :N_OBJ], RELU,
                                         bias=ob2[:, p2:p2 + 1], scale=1.0)
                ps = ps4p.tile([128, 512], F32, tag="ps4")
                nc.tensor.matmul(ps[:, :OUT_D], G2[:, 0, :], ow3[:, 0, :], start=True, stop=False)
                nc.tensor.matmul(ps[:, :OUT_D], G2[:, 1, :], ow3[:, 1, :], start=False, stop=True)
                ob = perb.tile([N_OBJ, OUT_D], F32, tag="ob")
                nc.vector.tensor_tensor(ob, ps[:, :OUT_D], ob3r, ADD)
                nc.sync.dma_start(out_d[b], ob)

            flat = [(b, ci, base, mc) for b in range(B_CORE)
                    for ci, (base, mc) in enumerate(chs)]
            n_flat = len(flat)
            sts = [dict() for _ in flat]

            # critical small weights first, then chunk-0/1 streams, then the
            # bulk weights split across both rings; loop prefetches 2 chunks
            nc.sync.dma_start(WF, WF_d[:])
            nc.sync.dma_start(WB[:, 0:256], WB_d[:, 0:256])        # rw1i
            nc.gpsimd.dma_start(PBt, PB_d[:])                      # As/Ar
            dmaA(*flat[0][:4], sts[0])
            nc.sync.dma_start(WB[:, 256:1280], WB_d[:, 256:1280])  # rw2,rw3
            dmaA(*flat[1][:4], sts[1])
            nc.gpsimd.dma_start(WB[:, 1280:BF_COLS], WB_d[:, 1280:BF_COLS])
            nc.gpsimd.dma_start(PF, PF_d[:])

            for k in range(n_flat):
                if k + 2 < n_flat:
                    dmaA(*flat[k + 2][:4], sts[k + 2])
                groups = stageA(*flat[k][:4], sts[k])
                pend = []
                if k >= 1:
                    pend += stageB(*flat[k - 1][:4], sts[k - 1])
                if k >= 2:
                    pend += stageC(*flat[k - 2][:4], sts[k - 2])
                for g in groups:
                    g()
                for f in pend:
                    f()
                if k >= 2 and flat[k - 2][1] == n_ch - 1:
                    obj_mlp(flat[k - 2][0])
            for fill in stageB(*flat[-1][:4], sts[-1]):
                fill()
            for fill in stageC(*flat[-2][:4], sts[-2]):
                fill()
            for fill in stageC(*flat[-1][:4], sts[-1]):
                fill()
            obj_mlp(flat[-1][0])

    nc.compile()
    return nc


def _prep_inputs(objects, sender_relations, receiver_relations, relation_info,
                 external_effect_info, rw1, rb1, rw2, rb2, rw3, rb3, rw4, rb4,
                 ow1, ob1, ow2, ob2, ow3, ob3):
    bf16 = ml_dtypes.bfloat16
    f32 = np.float32

    def a(x):
        return np.ascontiguousarray(np.asarray(x, dtype=f32))

    objects = a(objects); sender_relations = a(sender_relations)
    receiver_relations = a(receiver_relations); relation_info = a(relation_info)
    external_effect_info = a(external_effect_info)
    rw1, rb1, rw2, rb2, rw3, rb3, rw4, rb4 = map(a, (rw1, rb1, rw2, rb2, rw3, rb3, rw4, rb4))
    ow1, ob1, ow2, ob2, ow3, ob3 = map(a, (ow1, ob1, ow2, ob2, ow3, ob3))

    info_t_bf = np.ascontiguousarray(relation_info.transpose(0, 2, 1)).astype(bf16)
    s_bf = sender_relations.astype(bf16)
    r_bf = receiver_relations.astype(bf16)
    # [b, rel, obj] -> [b, rel%128, rel//128, obj] so each partition's
    # per-chunk DMA read is contiguous
    r_rel_t = np.ascontiguousarray(
        receiver_relations.transpose(0, 2, 1)
        .reshape(B, N_REL // 128, 128, N_OBJ)
        .transpose(0, 2, 1, 3)).astype(bf16)

    def fold2(w, out_dim):
        # [256, out] -> [128, 2, out] -> [128, 2*out]
        return np.ascontiguousarray(
            w.reshape(2, 128, out_dim).transpose(1, 0, 2).reshape(128, 2 * out_dim))

    wb = np.zeros((128, BF_COLS), dtype=f32)
    o, n = _BF["rw1i"]; wb[:64, o:o + n] = rw1[256:320]
    o, n = _BF["rw2"]; wb[:, o:o + n] = fold2(rw2, HID)
    o, n = _BF["rw3"]; wb[:, o:o + n] = fold2(rw3, HID)
    o, n = _BF["rw4"]; wb[:, o:o + n] = fold2(rw4, EFF_D)
    o, n = _BF["ow1e"]; wb[:, o:o + n] = np.ascontiguousarray(
        ow1[160:288].reshape(128, 2, 128).reshape(128, 256))
    o, n = _BF["ow2"]; wb[:, o:o + n] = fold2(ow2, HID)
    o, n = _BF["ow3"]; wb[:, o:o + n] = fold2(ow3, OUT_D)
    o, n = _BF["B4"]; wb[:, o:o + n] = np.broadcast_to(np.tile(rb4, 4)[None, :], (128, 512))
    wb = wb.astype(bf16)

    wf = np.zeros((128, F32_COLS), dtype=f32)
    o, n = _F32["rb1"]; wf[:, o:o + n] = rb1.reshape(2, 128).T
    o, n = _F32["rb2"]; wf[:, o:o + n] = rb2.reshape(2, 128).T
    o, n = _F32["rb3"]; wf[:, o:o + n] = rb3.reshape(2, 128).T
    o, n = _F32["ob2"]; wf[:, o:o + n] = ob2.reshape(2, 128).T
    o, n = _F32["ob3r"]; wf[:, o:o + n] = np.broadcast_to(ob3[None, :], (128, OUT_D))

    in_maps = []
    for c in range(N_CORES):
        sl = slice(c * B_CORE, (c + 1) * B_CORE)
        m = {
            "wblob_bf": wb,
            "wblob_f32": wf,
            "s_rel": s_bf[sl],
            "r_rel": r_bf[sl],
            "info_t": info_t_bf[sl],
            "r_rel_t": r_rel_t[sl],
        }
        pbl = np.zeros((128, PB_COLS), dtype=f32)
        pf = np.zeros((128, PF_COLS), dtype=f32)
        for bi in range(B_CORE):
            O = objects[c * B_CORE + bi]
            X = external_effect_info[c * B_CORE + bi]
            As = O @ rw1[0:128]
            Arr = O @ rw1[128:256]
            o, n = _PB[f"As{bi}"]; pbl[:, o:o + n] = As
            o, n = _PB[f"Ar{bi}"]; pbl[:, o:o + n] = Arr
            g1 = ow1[0:128].T @ O.T + ow1[128:160].T @ X.T + ob1[:, None]
            o, n = _PF[f"G1pre{bi}"]
            pf[:, o:o + n] = np.ascontiguousarray(
                g1.reshape(2, 128, 128).transpose(1, 0, 2).reshape(128, 256))
        m["pblob_bf"] = pbl.astype(bf16)
        m["pblob_f32"] = pf
        in_maps.append(m)
    return in_maps


def run(in_maps, **spmd_kwargs):
    from concourse.bass_utils import run_bass_kernel_spmd

    if "nc" not in _CACHE:
        _CACHE["nc"] = build_kernel()
    return run_bass_kernel_spmd(_CACHE["nc"], in_maps,
                                core_ids=list(range(N_CORES)), **spmd_kwargs)


def kernel(**inputs) -> np.ndarray:
    in_maps = _prep_inputs(**inputs)
    res = run(in_maps)
    out = np.concatenate([r["out"].reshape(-1, OUT_D) for r in res.results], axis=0)
    return np.ascontiguousarray(out, dtype=np.float32)


# revision 25
# speedup vs baseline: 1.1871x; 1.0020x over previous
"""Trainium2 Bass kernel for BasicInteractionNetworkModule.

Data-parallel over batch (B=16) across 8 NeuronCores, 2 batches/core.

Math (per batch b):
  senders   = S^T @ O          [R, 128]   (S = sender_relations [128, R])
  receivers = R_rel^T @ O      [R, 128]
  rel_x = [senders, receivers, info]   [R, 320]
  h = relu-MLP(rel_x): 320 -> 256 -> 256 -> 256 -> 128 (relu after every layer)
  eff_recv = R_rel @ effects   [128, 128]
  obj_x = [O, ext, eff_recv]   [128, 288]
  out = relu-MLP2(obj_x): 288 -> 256 -> 256 -> 128 (no final relu)

Device strategy: relation-MLP activations feature-major (partition = feature)
so every layer is out^T = W^T @ H^T with the moving operand streaming 512-col
slabs at 1 col/cycle, all in bf16. Layer-1 folds the sender/receiver
projections via host-precomputed A_s = O @ rw1[:128], A_r = O @ rw1[128:256].
Three-stage software pipeline per chunk k: A(k)=L1/L2/L3, B(k-1)=L4,
C(k-2)=aggregation, so every PSUM-evac -> stationary-reload handoff has a
full chunk of slack. Each 512-col PSUM slab is one bank (psB rotates 5).
L4's bias is folded into the evacuation (DVE add, ACT relu). The aggregation
accumulates all 127 rel-blocks of a batch into a persistent PSUM region.
The object MLP's objects/ext contributions are host-precomputed (G1pre).
"""

import numpy as np
import ml_dtypes

B, N_OBJ, N_REL = 16, 128, 16256
OBJ_D, REL_D, EFF_D, EXT_D, OUT_D = 128, 64, 128, 32, 128
HID = 256
N_CORES = 8
B_CORE = B // N_CORES  # 2
M_CHUNK = 1024

_CACHE = {}

# column offsets in the shared bf16 weight blob
_BF = {}
_off = 0
for _name, _n in [("rw1i", 256), ("rw2", 512), ("rw3", 512), ("rw4", 256),
                  ("ow1e", 256), ("ow2", 512), ("ow3", 256), ("B4", 512)]:
    _BF[_name] = (_off, _n)
    _off += _n
BF_COLS = _off
# per-core bf16 blob: As/Ar per batch
_PB = {}
_off = 0
for _name, _n in [("As0", 256), ("Ar0", 256), ("As1", 256), ("Ar1", 256)]:
    _PB[_name] = (_off, _n)
    _off += _n
PB_COLS = _off
# shared f32 blob
_F32 = {}
_off = 0
for _name, _n in [("rb1", 2), ("rb2", 2), ("rb3", 2), ("ob2", 2), ("ob3r", 128)]:
    _F32[_name] = (_off, _n)
    _off += _n
F32_COLS = _off
# per-core f32 blob: G1pre per batch [128, 2, 128]
_PF = {"G1pre0": (0, 256), "G1pre1": (256, 256)}
PF_COLS = 512


def _chunks():
    out = []
    base = 0
    while base < N_REL:
        mc = min(M_CHUNK, N_REL - base)
        out.append((base, mc))
        base += mc
    return out


def _mtiles(mc):
    out = []
    base = 0
    while base < mc:
        n = min(512, mc - base)
        out.append((base, n))
        base += n
    return out


def build_kernel():
    from concourse import bacc
    import concourse.mybir as mybir
    import concourse.tile as tile

    F32 = mybir.dt.float32
    BF16 = mybir.dt.bfloat16
    RELU = mybir.ActivationFunctionType.Relu
    ADD = mybir.AluOpType.add
    MAX = mybir.AluOpType.max
    MULT = mybir.AluOpType.mult

    nc = bacc.Bacc(None)

    S_d = nc.dram_tensor("s_rel", [B_CORE, N_OBJ, N_REL], BF16, kind="ExternalInput")
    R_d = nc.dram_tensor("r_rel", [B_CORE, N_OBJ, N_REL], BF16, kind="ExternalInput")
    IT_d = nc.dram_tensor("info_t", [B_CORE, REL_D, N_REL], BF16, kind="ExternalInput")
    # receiver_relations pre-permuted on host: [b, p, s, o] = R^T[b, s*128+p, o]
    # so each partition's per-chunk read is one contiguous run
    RT_d = nc.dram_tensor("r_rel_t", [B_CORE, 128, N_REL // 128, N_OBJ], BF16,
                          kind="ExternalInput")

    WB_d = nc.dram_tensor("wblob_bf", [128, BF_COLS], BF16, kind="ExternalInput")
    PB_d = nc.dram_tensor("pblob_bf", [128, PB_COLS], BF16, kind="ExternalInput")
    WF_d = nc.dram_tensor("wblob_f32", [128, F32_COLS], F32, kind="ExternalInput")
    PF_d = nc.dram_tensor("pblob_f32", [128, PF_COLS], F32, kind="ExternalInput")

    out_d = nc.dram_tensor("out", [B_CORE, N_OBJ, OUT_D], F32, kind="ExternalOutput")

    with tile.TileContext(nc) as tc:
        with (
            tc.tile_pool(name="wts", bufs=1) as wts,
            tc.tile_pool(name="perb", bufs=2) as perb,
            tc.tile_pool(name="cin", bufs=7) as cin,
            tc.tile_pool(name="acts", bufs=3) as acts,
            tc.tile_pool(name="psB", bufs=5, space="PSUM") as psB,
            tc.tile_pool(name="ps4", bufs=2, space="PSUM") as ps4p,
            tc.tile_pool(name="psE", bufs=1, space="PSUM") as psE,
        ):
            WB = wts.tile([128, BF_COLS], BF16)
            PBt = wts.tile([128, PB_COLS], BF16)
            WF = wts.tile([128, F32_COLS], F32)
            PF = wts.tile([128, PF_COLS], F32)

            def bf(name):
                o, n = _BF[name]
                return WB[:, o:o + n]

            def pb(name):
                o, n = _PB[name]
                return PBt[:, o:o + n]

            def f32(name):
                o, n = _F32[name]
                return WF[:, o:o + n]

            _o, _n = _BF["rw1i"]
            rw1i = WB[:64, _o:_o + _n].rearrange("p (two h) -> p two h", two=2)
            rw2 = bf("rw2").rearrange("p (two h) -> p two h", two=2)
            rw3 = bf("rw3").rearrange("p (two h) -> p two h", two=2)
            rw4 = bf("rw4").rearrange("p (two h) -> p two h", two=2)
            ow1e = bf("ow1e").rearrange("p (two h) -> p two h", two=2)
            ow2 = bf("ow2").rearrange("p (two h) -> p two h", two=2)
            ow3 = bf("ow3").rearrange("p (two h) -> p two h", two=2)
            B4 = bf("B4")
            rb1 = f32("rb1")
            rb2 = f32("rb2")
            rb3 = f32("rb3")
            ob2 = f32("ob2")
            ob3r = f32("ob3r")
            As = [pb("As0"), pb("As1")]
            Ar = [pb("Ar0"), pb("Ar1")]
            G1pre = [PF[:, 0:256].rearrange("p (two h) -> p two h", two=2),
                     PF[:, 256:512].rearrange("p (two h) -> p two h", two=2)]

            chs = _chunks()
            n_ch = len(chs)
            effaccs = {}
            effacc_all = psE.tile([128, B_CORE * N_OBJ], F32, tag="effacc")
            for b in range(B_CORE):
                effaccs[b] = effacc_all[:, b * N_OBJ:(b + 1) * N_OBJ]

            def dmaA(b, ci, base, mc, st):
                ns = mc // 128
                S_c = cin.tile([N_OBJ, M_CHUNK], BF16, tag="S_c")
                R_c = cin.tile([N_OBJ, M_CHUNK], BF16, tag="R_c")
                I_c = cin.tile([REL_D, M_CHUNK], BF16, tag="I_c")
                RT_c = cin.tile([128, M_CHUNK // 128, N_OBJ], BF16, tag="RT_c")
                nc.sync.dma_start(S_c[:, :mc], S_d[b, :, base:base + mc])
                nc.sync.dma_start(I_c[:, :mc], IT_d[b, :, base:base + mc])
                nc.gpsimd.dma_start(R_c[:, :mc], R_d[b, :, base:base + mc])
                s0 = base // 128
                nc.gpsimd.dma_start(RT_c[:, :ns, :], RT_d[b, :, s0:s0 + ns, :])
                st['S_c'] = S_c; st['R_c'] = R_c; st['I_c'] = I_c; st['RT_c'] = RT_c

            def stageA(b, ci, base, mc, st):
                """Returns a list of slab-group closures (stall-prone)."""
                S_c = st['S_c']; R_c = st['R_c']; I_c = st['I_c']
                H1 = acts.tile([128, 2, M_CHUNK], BF16, tag="H1")
                H2 = acts.tile([128, 2, M_CHUNK], BF16, tag="H2")
                H3 = acts.tile([128, 2, M_CHUNK], BF16, tag="H3")
                st['H3'] = H3
                groups = []

                h0, h1 = slice(0, 128), slice(128, 256)

                def l1(sl, n):
                    # both halves' accumulation chains interleaved over two
                    # psum banks: a bank gets a full stream-time between its
                    # chained read-modify-writes, hiding the write drain
                    def go():
                        pa = psB.tile([128, 512], F32, tag="ps")
                        pb2 = psB.tile([128, 512], F32, tag="ps")
                        nc.tensor.matmul(pa[:, :n], rw1i[:, 0, :], I_c[:, sl], start=True, stop=False)
                        nc.tensor.matmul(pb2[:, :n], rw1i[:, 1, :], I_c[:, sl], start=True, stop=False)
                        nc.tensor.matmul(pa[:, :n], As[b][:, h0], S_c[:, sl], start=False, stop=False)
                        nc.tensor.matmul(pb2[:, :n], As[b][:, h1], S_c[:, sl], start=False, stop=False)
                        nc.tensor.matmul(pa[:, :n], Ar[b][:, h0], R_c[:, sl], start=False, stop=True)
                        nc.tensor.matmul(pb2[:, :n], Ar[b][:, h1], R_c[:, sl], start=False, stop=True)
                        nc.scalar.activation(H1[:, 0, sl], pa[:, :n], RELU,
                                             bias=rb1[:, 0:1], scale=1.0)
                        nc.scalar.activation(H1[:, 1, sl], pb2[:, :n], RELU,
                                             bias=rb1[:, 1:2], scale=1.0)
                    return go

                def l23(sl, n, w, Hin, Hout, rb):
                    def go():
                        pa = psB.tile([128, 512], F32, tag="ps")
                        pb2 = psB.tile([128, 512], F32, tag="ps")
                        nc.tensor.matmul(pa[:, :n], w[:, 0, h0], Hin[:, 0, sl], start=True, stop=False)
                        nc.tensor.matmul(pb2[:, :n], w[:, 0, h1], Hin[:, 0, sl], start=True, stop=False)
                        nc.tensor.matmul(pa[:, :n], w[:, 1, h0], Hin[:, 1, sl], start=False, stop=True)
                        nc.tensor.matmul(pb2[:, :n], w[:, 1, h1], Hin[:, 1, sl], start=False, stop=True)
                        nc.scalar.activation(Hout[:, 0, sl], pa[:, :n], RELU,
                                             bias=rb[:, 0:1], scale=1.0)
                        nc.vector.tensor_scalar(Hout[:, 1, sl], pb2[:, :n],
                                                rb[:, 1:2], 0.0, ADD, MAX)
                    return go

                for mt, n in _mtiles(mc):
                    groups.append(l1(slice(mt, mt + n), n))
                for mt, n in _mtiles(mc):
                    groups.append(l23(slice(mt, mt + n), n, rw2, H1, H2, rb2))
                for mt, n in _mtiles(mc):
                    groups.append(l23(slice(mt, mt + n), n, rw3, H2, H3, rb3))
                return groups

            def stageB(b, ci, base, mc, st):
                """Returns closures of slack-rich L4 work (never stalls)."""
                ns = mc // 128
                H3 = st['H3']
                E3t = acts.tile([128, M_CHUNK], BF16, tag="E3t")
                E3 = acts.tile([128, M_CHUNK], BF16, tag="E3")
                st['E3'] = E3
                fills = []
                for g in range(0, ns, 4):
                    ge = min(g + 4, ns)

                    def l4(g, ge):
                        def go():
                            span = (ge - g) * 128
                            gsl = slice(g * 128, g * 128 + span)
                            ps4 = ps4p.tile([128, 512], F32, tag="ps4")
                            for sj in range(g, ge):
                                sl = slice(sj * 128, (sj + 1) * 128)
                                psl = slice((sj - g) * 128, (sj - g + 1) * 128)
                                nc.tensor.matmul(ps4[:, psl], H3[:, 0, sl], rw4[:, 0, :],
                                                 start=True, stop=False, skip_group_check=True)
                                nc.tensor.matmul(ps4[:, psl], H3[:, 1, sl], rw4[:, 1, :],
                                                 start=False, stop=True, skip_group_check=True)
                            nc.vector.scalar_tensor_tensor(E3t[:, gsl], ps4[:, :span], 1.0,
                                                           B4[:, :span], MULT, ADD)
                            nc.vector.tensor_scalar_max(E3[:, gsl], E3t[:, gsl], 0.0)
                        return go

                    fills.append(l4(g, ge))
                return fills

            def stageC(b, ci, base, mc, st):
                """Returns closures of slack-rich aggregation work."""
                ns = mc // 128
                E3 = st['E3']; RT_c = st['RT_c']
                effacc = effaccs[b]

                def agg(s0, s1):
                    def go():
                        for sj in range(s0, s1):
                            nc.tensor.matmul(effacc, E3[:, sj * 128:(sj + 1) * 128],
                                             RT_c[:, sj, :],
                                             start=(ci == 0 and sj == 0),
                                             stop=(ci == n_ch - 1 and sj == ns - 1),
                                             skip_group_check=True)
                    return go

                mid = (ns + 1) // 2
                return [agg(0, mid), agg(mid, ns)]

            def obj_mlp(b):
                effacc = effaccs[b]
                effTb = perb.tile([128, N_OBJ], BF16, tag="effTb")
                nc.scalar.copy(effTb, effacc)
                G1 = perb.tile([128, 2, N_OBJ], BF16, tag="G1")
                G2 = perb.tile([128, 2, N_OBJ], BF16, tag="G2")
                Gt = perb.tile([128, N_OBJ], F32, tag="Gt")
                for p2 in range(2):
                    ps = ps4p.tile([128, 512], F32, tag="ps4")
                    nc.tensor.matmul(ps[:, :N_OBJ], ow1e[:, p2, :], effTb, start=True, stop=True)
                    nc.vector.scalar_tensor_tensor(Gt, ps[:, :N_OBJ], 1.0,
                                                   G1pre[b][:, p2, :], MULT, ADD)
                    nc.scalar.activation(G1[:, p2, :], Gt, RELU)
                for p2 in range(2):
                    ps = ps4p.tile([128, 512], F32, tag="ps4")
                    h = slice(p2 * 128, (p2 + 1) * 128)
                    nc.tensor.matmul(ps[:, :N_OBJ], ow2[:, 0, h], G1[:, 0, :], start=True, stop=False)
                    nc.tensor.matmul(ps[:, :N_OBJ], ow2[:, 1, h], G1[:, 1, :], start=False, stop=True)
                    nc.scalar.activation(G2[:, p2, :], ps[:, :N_OBJ], RELU,
                                         bias=ob2[:, p2:p2 + 1], scale=1.0)
                ps = ps4p.tile([128, 512], F32, tag="ps4")
                nc.tensor.matmul(ps[:, :OUT_D], G2[:, 0, :], ow3[:, 0, :], start=True, stop=False)
                nc.tensor.matmul(ps[:, :OUT_D], G2[:, 1, :], ow3[:, 1, :], start=False, stop=True)
                ob = perb.tile([N_OBJ, OUT_D], F32, tag="ob")
                nc.vector.tensor_tensor(ob, ps[:, :OUT_D], ob3r, ADD)
                nc.sync.dma_start(out_d[b], ob)

            flat = [(b, ci, base, mc) for b in range(B_CORE)
                    for ci, (base, mc) in enumerate(chs)]
            n_flat = len(flat)
            sts = [dict() for _ in flat]

            # critical small weights first, then chunk-0/1 streams, then the
            # bulk weights split across both rings; loop prefetches 2 chunks
            nc.sync.dma_start(WF, WF_d[:])
            nc.sync.dma_start(WB[:, 0:256], WB_d[:, 0:256])        # rw1i
            nc.gpsimd.dma_start(PBt, PB_d[:])                      # As/Ar
            dmaA(*flat[0][:4], sts[0])
            nc.sync.dma_start(WB[:, 256:1280], WB_d[:, 256:1280])  # rw2,rw3
            dmaA(*flat[1][:4], sts[1])
            nc.gpsimd.dma_start(WB[:, 1280:BF_COLS], WB_d[:, 1280:BF_COLS])
            nc.gpsimd.dma_start(PF, PF_d[:])
            dmaA(*flat[2][:4], sts[2])

            for k in range(n_flat):
                if k + 3 < n_flat:
                    dmaA(*flat[k + 3][:4], sts[k + 3])
                groups = stageA(*flat[k][:4], sts[k])
                pend = []
                if k >= 1:
                    pend += stageB(*flat[k - 1][:4], sts[k - 1])
                if k >= 2:
                    pend += stageC(*flat[k - 2][:4], sts[k - 2])
                for g in groups:
                    g()
                for f in pend:
                    f()
                if k >= 2 and flat[k - 2][1] == n_ch - 1:
                    obj_mlp(flat[k - 2][0])
            for fill in stageB(*flat[-1][:4], sts[-1]):
                fill()
            for fill in stageC(*flat[-2][:4], sts[-2]):
                fill()
            for fill in stageC(*flat[-1][:4], sts[-1]):
                fill()
            obj_mlp(flat[-1][0])

    nc.compile()
    return nc


def _prep_inputs(objects, sender_relations, receiver_relations, relation_info,
                 external_effect_info, rw1, rb1, rw2, rb2, rw3, rb3, rw4, rb4,
                 ow1, ob1, ow2, ob2, ow3, ob3):
    bf16 = ml_dtypes.bfloat16
    f32 = np.float32

    def a(x):
        return np.ascontiguousarray(np.asarray(x, dtype=f32))

    objects = a(objects); sender_relations = a(sender_relations)
    receiver_relations = a(receiver_relations); relation_info = a(relation_info)
    external_effect_info = a(external_effect_info)
    rw1, rb1, rw2, rb2, rw3, rb3, rw4, rb4 = map(a, (rw1, rb1, rw2, rb2, rw3, rb3, rw4, rb4))
    ow1, ob1, ow2, ob2, ow3, ob3 = map(a, (ow1, ob1, ow2, ob2, ow3, ob3))

    info_t_bf = np.ascontiguousarray(relation_info.transpose(0, 2, 1)).astype(bf16)
    s_bf = sender_relations.astype(bf16)
    r_bf = receiver_relations.astype(bf16)
    # [b, rel, obj] -> [b, rel%128, rel//128, obj] so each partition's
    # per-chunk DMA read is contiguous
    r_rel_t = np.ascontiguousarray(
        receiver_relations.transpose(0, 2, 1)
        .reshape(B, N_REL // 128, 128, N_OBJ)
        .transpose(0, 2, 1, 3)).astype(bf16)

    def fold2(w, out_dim):
        # [256, out] -> [128, 2, out] -> [128, 2*out]
        return np.ascontiguousarray(
            w.reshape(2, 128, out_dim).transpose(1, 0, 2).reshape(128, 2 * out_dim))

    wb = np.zeros((128, BF_COLS), dtype=f32)
    o, n = _BF["rw1i"]; wb[:64, o:o + n] = rw1[256:320]
    o, n = _BF["rw2"]; wb[:, o:o + n] = fold2(rw2, HID)
    o, n = _BF["rw3"]; wb[:, o:o + n] = fold2(rw3, HID)
    o, n = _BF["rw4"]; wb[:, o:o + n] = fold2(rw4, EFF_D)
    o, n = _BF["ow1e"]; wb[:, o:o + n] = np.ascontiguousarray(
        ow1[160:288].reshape(128, 2, 128).reshape(128, 256))
    o, n = _BF["ow2"]; wb[:, o:o + n] = fold2(ow2, HID)
    o, n = _BF["ow3"]; wb[:, o:o + n] = fold2(ow3, OUT_D)
    o, n = _BF["B4"]; wb[:, o:o + n] = np.broadcast_to(np.tile(rb4, 4)[None, :], (128, 512))
    wb = wb.astype(bf16)

    wf = np.zeros((128, F32_COLS), dtype=f32)
    o, n = _F32["rb1"]; wf[:, o:o + n] = rb1.reshape(2, 128).T
    o, n = _F32["rb2"]; wf[:, o:o + n] = rb2.reshape(2, 128).T
    o, n = _F32["rb3"]; wf[:, o:o + n] = rb3.reshape(2, 128).T
    o, n = _F32["ob2"]; wf[:, o:o + n] = ob2.reshape(2, 128).T
    o, n = _F32["ob3r"]; wf[:, o:o + n] = np.broadcast_to(ob3[None, :], (128, OUT_D))

    in_maps = []
    for c in range(N_CORES):
        sl = slice(c * B_CORE, (c + 1) * B_CORE)
        m = {
            "wblob_bf": wb,
            "wblob_f32": wf,
            "s_rel": s_bf[sl],
            "r_rel": r_bf[sl],
            "info_t": info_t_bf[sl],
            "r_rel_t": r_rel_t[sl],
        }
        pbl = np.zeros((128, PB_COLS), dtype=f32)
        pf = np.zeros((128, PF_COLS), dtype=f32)
        for bi in range(B_CORE):
            O = objects[c * B_CORE + bi]
            X = external_effect_info[c * B_CORE + bi]
            As = O @ rw1[0:128]
            Arr = O @ rw1[128:256]
            o, n = _PB[f"As{bi}"]; pbl[:, o:o + n] = As
            o, n = _PB[f"Ar{bi}"]; pbl[:, o:o + n] = Arr
            g1 = ow1[0:128].T @ O.T + ow1[128:160].T @ X.T + ob1[:, None]
            o, n = _PF[f"G1pre{bi}"]
            pf[:, o:o + n] = np.ascontiguousarray(
                g1.reshape(2, 128, 128).transpose(1, 0, 2).reshape(128, 256))
        m["pblob_bf"] = pbl.astype(bf16)
        m["pblob_f32"] = pf
        in_maps.append(m)
    return in_maps


def run(in_maps, **spmd_kwargs):
    from concourse.bass_utils import run_bass_kernel_spmd

    if "nc" not in _CACHE:
        _CACHE["nc"] = build_kernel()
    return run_bass_kernel_spmd(_CACHE["nc"], in_maps,
                                core_ids=list(range(N_CORES)), **spmd_kwargs)


def kernel(**inputs) -> np.ndarray:
    in_maps = _prep_inputs(**inputs)
    res = run(in_maps)
    out = np.concatenate([r["out"].reshape(-1, OUT_D) for r in res.results], axis=0)
    return np.ascontiguousarray(out, dtype=np.float32)
